# revision 1
# baseline (speedup 1.0000x reference)
"""Trainium2 Bass kernel for the pairwise contact-map decoder.

Reference computation (per batch b):
    tmp[b,i,c,h] = sum_a z[b,i,a] * W1[(a,c),h]
    h1[b,i,j,h]  = relu(sum_c tmp[b,i,c,h] * z[b,j,c] + b1[h])
    h2[b,i,j,k]  = relu(sum_h h1[b,i,j,h] * W2[h,k] + b2[k])
    logit[b,i,j] = (sum_k h2[b,i,j,k] * W3[k,0] + b3) * motif[b,i] * motif[b,j]
    cmap         = sigmoid(logit)

Sharding: 8 cores, each takes 128 contiguous i-rows of one batch
(core = 2*b + half). Weights and z[b] are replicated per core.

On-core dataflow:
  stage A (float32r matmuls, full PE rate, ~1e-4 err): tmp2[i, c, h] =
           ziT.T @ W1 (viewed (a, (c,h))), staged to an fp16 DRAM scratch
           with an extra c-row holding b1 (bias folded via K=33).
  per i-pair (fp16 matmul inputs, fp32 PSUM accumulate; fp16 stationaries
  get fast weight loads that overlap the matmuls):
              stage B  h1T[h,(i,j)] = tmp2_i.T @ zTx  (K=33 includes bias)
              stage C  h2T[k,(i,j)] accumulate over 4 h-chunks of W2
              stage D  logits strip (1, 512) via W3 chunks
  Stage C/D of pair p runs after stage B of pair p+1 (software pipeline)
  so the in-order PE never waits on PSUM evictions.
  epilogue: outer motif mask via a K=1 matmul, mask-mul, sigmoid, DMA out,
  in row-halves as soon as their logits land.
End-to-end max rel err vs the fp32 reference: ~7e-4.
"""

import numpy as np

import concourse.bass as bass
import concourse.mybir as mybir
import concourse.tile as tile
from concourse import bacc
from concourse.bass_utils import run_bass_kernel_spmd

B, N, D, H = 4, 256, 32, 512
DT = mybir.dt
F32, F32R, F16 = DT.float32, DT.float32r, DT.float16
AF = mybir.ActivationFunctionType
ALU = mybir.AluOpType
NCORES = 8
ROWS = 128  # i-rows per core
NPAIR = ROWS // 2

_cached_nc = {}


from contextlib import nullcontext as _nullcontext


def _r(ap):
    return ap.bitcast(F32R)


def _build(reps=1):
    nc = bacc.Bacc("TRN2", target_bir_lowering=False, debug=False, num_devices=NCORES)

    ziT = nc.dram_tensor("ziT", [D, ROWS], F32, kind="ExternalInput")
    zTx = nc.dram_tensor("zTx", [D + 1, N], F32, kind="ExternalInput")
    W1 = nc.dram_tensor("W1", [D * D, H], F32, kind="ExternalInput")
    W2 = nc.dram_tensor("W2", [H, H // 2], F32, kind="ExternalInput")
    W3 = nc.dram_tensor("W3", [H // 2, 1], F32, kind="ExternalInput")
    b1 = nc.dram_tensor("b1", [H], F32, kind="ExternalInput")
    b2 = nc.dram_tensor("b2", [H // 2], F32, kind="ExternalInput")
    b3 = nc.dram_tensor("b3", [1], F32, kind="ExternalInput")
    mi = nc.dram_tensor("mi", [1, ROWS], F32, kind="ExternalInput")
    mj = nc.dram_tensor("mj", [1, N], F32, kind="ExternalInput")
    logits_o = nc.dram_tensor("logits", [ROWS, N], F32, kind="ExternalOutput")
    cmap_o = nc.dram_tensor("cmap", [ROWS, N], F32, kind="ExternalOutput")
    # scratch holding tmp2 transposed per i: (i, c, h) with c=32 rows + b1 row
    tmp2x = nc.dram_tensor("tmp2x", [ROWS, D + 1, H], F16)

    with tile.TileContext(nc) as tc:
        with (
            tc.tile_pool(name="const", bufs=1) as cp,
            tc.tile_pool(name="work", bufs=3) as wp,
            tc.tile_pool(name="ps", bufs=2, space="PSUM") as ps,
        ):
          with tc.For_i(0, reps, 1) if reps > 1 else _nullcontext():
              # ---------- persistent loads ----------
              ziT_s = cp.tile([D, ROWS], F32R)
              nc.sync.dma_start(ziT_s[:], _r(ziT.ap()))
              W1v = _r(W1.ap().rearrange("(a c) h -> a c h", a=D))
              W1_s = cp.tile([D, D, H], F32R)
              nc.sync.dma_start(W1_s[:, 0:4, :], W1v[:, 0:4, :])
              nc.sync.dma_start(W1_s[:, 4:8, :], W1v[:, 4:8, :])
              for q in range(1, 4):
                  nc.sync.dma_start(W1_s[:, 8 * q : 8 * (q + 1), :], W1v[:, 8 * q : 8 * (q + 1), :])
              zTx_s = cp.tile([D + 1, N], F16)
              nc.gpsimd.dma_start(zTx_s[:], zTx.ap())
              W2_s = cp.tile([128, 4, 256], F16)
              nc.gpsimd.dma_start(W2_s[:], W2.ap().rearrange("(c p) k -> p c k", c=4))
              W3_s = cp.tile([128, 2], F16)
              nc.gpsimd.dma_start(W3_s[:], W3.ap().rearrange("(c p) o -> p (c o)", c=2))
              b2_s = cp.tile([128, 2], F32)
              nc.sync.dma_start(b2_s[:], b2.ap().rearrange("(c p) -> p c", c=2))
              b3_s = cp.tile([1, 1], F32)
              nc.sync.dma_start(b3_s[:], b3.ap().unsqueeze(0))
              mi_s = cp.tile([1, ROWS], F32R)
              nc.sync.dma_start(mi_s[:], _r(mi.ap()))
              mj_s = cp.tile([1, N], F32R)
              nc.sync.dma_start(mj_s[:], _r(mj.ap()))
              logits_sb = cp.tile([ROWS, N], F32)

              # bias row of the scratch: tmp2x[:, D, :] = b1 for every i
              # (DRAM->DRAM casting broadcast; gpsimd is the only caster)
              nc.gpsimd.dma_start(
                  tmp2x.ap()[:, D, :],
                  b1.ap().unsqueeze(0).broadcast_to([ROWS, H]),
              )

              # psM only needs mi/mj: compute the outer mask up front so the
              # epilogue isn't serialized behind a matmul at the tail
              psM = ps.tile([ROWS, N], F32, tag="m", bufs=1)
              nc.tensor.matmul(psM[:], mi_s[:], mj_s[:], start=True, stop=True)
              mask_sb = cp.tile([ROWS, N], F32)
              nc.vector.tensor_copy(mask_sb[:], psM[:])

              # ---------- stage A: tmp2x[:, c, :] ----------
              # W1 is streamed per c-chunk (no big upfront load); the output
              # DMA is split into i-halves so early pairs' reads only wait on
              # the top half of the scratch.
              # evict two c-chunks into one tile and write them with a
              # single DMA: HWDGE queue slots (~625ns each) dominate the
              # 182ns transfers, so fewer/bigger DMAs win
              sbA = None
              for n in range(D):
                  psA = ps.tile([ROWS, H], F32, tag="ac")
                  nc.tensor.matmul(psA[:], ziT_s[:], W1_s[:, n, :], start=True, stop=True)
                  if n % 2 == 0:
                      sbA = wp.tile([ROWS, 2, H], F16, tag="sa")
                      nc.vector.tensor_copy(sbA[:, 0, :], psA[:])
                  else:
                      nc.scalar.copy(sbA[:, 1, :], psA[:])
                      nc.sync.dma_start(tmp2x.ap()[:, n - 1 : n + 1, :], sbA[:])

              # ---------- main loop over i-pairs (software-pipelined) ----------
              # Stage C/D of pair p is emitted after stage B of pair p+1 so the
              # PE never waits on the DVE relu-eviction of h1T (in-order PE
              # stream would otherwise stall ~1.5us per pair).
              def stage_B(p):
                  tp = wp.tile([D + 1, 2, H], F16, tag="tp")
                  nc.scalar.dma_start(
                      tp[:], tmp2x.ap()[2 * p : 2 * p + 2].rearrange("i c h -> c i h")
                  )
                  h1T = wp.tile([128, 4, 2 * N], F16, tag="h1")
                  for i in range(2):
                      psB = ps.tile([128, 4, N], F32, tag="b")
                      for hc in range(4):
                          nc.tensor.matmul(
                              psB[:, hc, :],
                              tp[:, i, hc * 128 : (hc + 1) * 128],
                              zTx_s[:],
                              start=(hc % 2 == 0),
                              stop=(hc % 2 == 1),
                          )
                      # relu; bias already folded in via the K=33 ones row
                      nc.vector.tensor_scalar(
                          h1T[:, :, i * N : (i + 1) * N], psB[:], 0.0, None, ALU.max
                      )
                  return h1T

              def stage_CD(p, h1T):
                  h2T = wp.tile([128, 2, 2 * N], F16, tag="h2")
                  for kc in range(2):
                      psC = ps.tile([128, 2 * N], F32, tag="ac")
                      for hc in range(4):
                          nc.tensor.matmul(
                              psC[:],
                              W2_s[:, hc, kc * 128 : (kc + 1) * 128],
                              h1T[:, hc, :],
                              start=(hc == 0),
                              stop=(hc == 3),
                          )
                      nc.scalar.activation(
                          h2T[:, kc, :], psC[:], AF.Relu, bias=b2_s[:, kc : kc + 1]
                      )
                  psD = ps.tile([1, 2 * N], F32, tag="d", bufs=1)
                  nc.tensor.matmul(psD[:], W3_s[:, 0:1], h2T[:, 0, :], start=True, stop=False)
                  nc.tensor.matmul(psD[:], W3_s[:, 1:2], h2T[:, 1, :], start=False, stop=True)
                  strip = wp.tile([1, 2 * N], F32, tag="st")
                  nc.scalar.activation(strip[:], psD[:], AF.Identity, bias=b3_s[:])
                  nc.scalar.dma_start(logits_sb[2 * p : 2 * p + 2, :], strip[:])

              # epilogue runs in row-halves as soon as their logits land
              mlog = cp.tile([ROWS, N], F32)
              cmap_sb = cp.tile([ROWS, N], F32)

              def epilogue_half(h):
                  rows = slice(64 * h, 64 * (h + 1))
                  nc.vector.tensor_mul(mlog[rows, :], logits_sb[rows, :], mask_sb[rows, :])
                  nc.sync.dma_start(logits_o.ap()[rows, :], mlog[rows, :])
                  nc.scalar.activation(cmap_sb[rows, :], mlog[rows, :], AF.Sigmoid)
                  nc.sync.dma_start(cmap_o.ap()[rows, :], cmap_sb[rows, :])

              prev = None
              for p in range(NPAIR):
                  h1T_p = stage_B(p)
                  if prev is not None:
                      stage_CD(*prev)
                      if prev[0] == 31:
                          epilogue_half(0)
                  prev = (p, h1T_p)
              stage_CD(*prev)
              epilogue_half(1)

    nc.compile()
    return nc


def _in_maps(z, motif_mask, W1, b1, W2, b2, W3, b3):
    z = np.ascontiguousarray(np.asarray(z, dtype=np.float32))
    motif_mask = np.asarray(motif_mask, dtype=np.float32)
    W1 = np.ascontiguousarray(np.asarray(W1, dtype=np.float32)).reshape(D * D, H)
    W2 = np.ascontiguousarray(np.asarray(W2, dtype=np.float32)).reshape(H, H // 2)
    W3 = np.ascontiguousarray(np.asarray(W3, dtype=np.float32)).reshape(H // 2, 1)
    b1 = np.ascontiguousarray(np.asarray(b1, dtype=np.float32)).reshape(H)
    b2 = np.ascontiguousarray(np.asarray(b2, dtype=np.float32)).reshape(H // 2)
    b3 = np.ascontiguousarray(np.asarray(b3, dtype=np.float32)).reshape(1)
    maps = []
    for c in range(NCORES):
        b, half = divmod(c, 2)
        rows = slice(half * ROWS, (half + 1) * ROWS)
        zb = z[b]  # (N, D)
        zTx = np.concatenate([zb.T, np.ones((1, N), np.float32)], axis=0)
        maps.append(
            {
                "ziT": np.ascontiguousarray(zb[rows].T),
                "zTx": np.ascontiguousarray(zTx),
                "W1": W1,
                "W2": W2,
                "W3": W3,
                "b1": b1,
                "b2": b2,
                "b3": b3,
                "mi": np.ascontiguousarray(motif_mask[b, rows].reshape(1, ROWS)),
                "mj": np.ascontiguousarray(motif_mask[b].reshape(1, N)),
            }
        )
    return maps


def kernel(z, motif_mask, residue_mask, W1, b1, W2, b2, W3, b3):
    global _cached_nc
    if 1 not in _cached_nc:
        _cached_nc[1] = _build()
    nc = _cached_nc[1]

    maps = _in_maps(z, motif_mask, W1, b1, W2, b2, W3, b3)
    res = run_bass_kernel_spmd(nc, maps, list(range(NCORES)))

    logits = np.empty((B, N, N), np.float32)
    cmap = np.empty((B, N, N), np.float32)
    for c in range(NCORES):
        b, half = divmod(c, 2)
        rows = slice(half * ROWS, (half + 1) * ROWS)
        logits[b, rows] = res.results[c]["logits"]
        cmap[b, rows] = res.results[c]["cmap"]
    return cmap, logits



# revision 17
# speedup vs baseline: 1.0627x; 1.0627x over previous
"""Trainium2 Bass kernel for the pairwise contact-map decoder.

Reference computation (per batch b):
    tmp[b,i,c,h] = sum_a z[b,i,a] * W1[(a,c),h]
    h1[b,i,j,h]  = relu(sum_c tmp[b,i,c,h] * z[b,j,c] + b1[h])
    h2[b,i,j,k]  = relu(sum_h h1[b,i,j,h] * W2[h,k] + b2[k])
    logit[b,i,j] = (sum_k h2[b,i,j,k] * W3[k,0] + b3) * motif[b,i] * motif[b,j]
    cmap         = sigmoid(logit)

Key structural fact: the outer motif mask zeroes every (i,j) where either
index is masked out, making logit exactly 0 and cmap exactly sigmoid(0)=0.5
there.  So only the active submatrix (rows/cols with motif==1, ~50% each,
~25% of the grid) ever needs computing.  The host gathers active rows/cols,
the device computes the dense active block, and the host scatters results
into a 0 / 0.5 - filled full-size output.  Exact: masked entries match the
reference bit-for-bit, active entries follow the same fp16 path as before.

Sharding: 8 cores, core = 2*b + half; the two cores of a batch split its
active rows.  Compiled slab: ROWS=72 i-rows x NJ=144 j-cols (covers any
mask with <=144 active per batch; a full-size 128x256 variant is built
lazily as fallback for larger masks).

On-core dataflow (identical pipeline to the full-grid version):
  stage A (float32r matmuls): tmp2[i, c, h] staged to an fp16 DRAM scratch
           with an extra c-row holding b1 (bias folded via K=33).
  per i-pair: stage B  h1T[h,(i,j)] = tmp2_i.T @ zTx  (K=33 includes bias)
              stage C  h2T[k,(i,j)] accumulate over 4 h-chunks of W2
              stage D  logits strip (1, 2*NJ) via W3 chunks
  Stage C/D of pair p runs after stage B of pair p+1 (software pipeline).
  epilogue: sigmoid + DMA out in row-halves (no mask work on device).
"""

import numpy as np

import concourse.bass as bass
import concourse.mybir as mybir
import concourse.tile as tile
from concourse import bacc
from concourse.bass_utils import run_bass_kernel_spmd

B, N, D, H = 4, 256, 32, 512
DT = mybir.dt
F32, F32R, F16 = DT.float32, DT.float32r, DT.float16
AF = mybir.ActivationFunctionType
ALU = mybir.AluOpType
NCORES = 8
ROWS = 72   # padded active i-rows per core
NJ = 144    # padded active j-cols per batch

_cached_nc = {}


from contextlib import nullcontext as _nullcontext


def _r(ap):
    return ap.bitcast(F32R)


def _build(reps=1, rows=ROWS, nj=NJ):
    npair = rows // 2
    nc = bacc.Bacc("TRN2", target_bir_lowering=False, debug=False, num_devices=NCORES)

    ziT = nc.dram_tensor("ziT", [D, rows], F32, kind="ExternalInput")
    zTx = nc.dram_tensor("zTx", [D + 1, nj], F32, kind="ExternalInput")
    W1 = nc.dram_tensor("W1", [D * D, H], F32, kind="ExternalInput")
    W2 = nc.dram_tensor("W2", [H, H // 2], F32, kind="ExternalInput")
    W3 = nc.dram_tensor("W3", [H // 2, 1], F32, kind="ExternalInput")
    b1 = nc.dram_tensor("b1", [H], F32, kind="ExternalInput")
    b2 = nc.dram_tensor("b2", [H // 2], F32, kind="ExternalInput")
    b3 = nc.dram_tensor("b3", [1], F32, kind="ExternalInput")
    logits_o = nc.dram_tensor("logits", [rows, nj], F32, kind="ExternalOutput")
    cmap_o = nc.dram_tensor("cmap", [rows, nj], F32, kind="ExternalOutput")
    # c-major scratch: transpose lives on the DRAM-side write AP, reads are
    # clean strided loads (SBUF APs cannot cross partitions)
    tmp2xT = nc.dram_tensor("tmp2xT", [D, rows, H], F16)

    with tile.TileContext(nc) as tc:
        with (
            tc.tile_pool(name="const", bufs=1) as cp,
            tc.tile_pool(name="work", bufs=3) as wp,
            tc.tile_pool(name="ps", bufs=2, space="PSUM") as ps,
        ):
          with tc.For_i(0, reps, 1) if reps > 1 else _nullcontext():
              # ---------- persistent loads ----------
              ziT_s = cp.tile([D, rows], F32R)
              nc.sync.dma_start(ziT_s[:], _r(ziT.ap()))
              W1v = _r(W1.ap().rearrange("(a c) h -> a c h", a=D))
              W1_s = cp.tile([D, D, H], F32R)
              nc.sync.dma_start(W1_s[:, 0:4, :], W1v[:, 0:4, :])
              nc.sync.dma_start(W1_s[:, 4:8, :], W1v[:, 4:8, :])
              for q in range(1, 4):
                  nc.sync.dma_start(W1_s[:, 8 * q : 8 * (q + 1), :], W1v[:, 8 * q : 8 * (q + 1), :])
              zTx_s = cp.tile([D + 1, nj], F16)
              nc.gpsimd.dma_start(zTx_s[:], zTx.ap())
              W2_s = cp.tile([128, 4, 256], F16)
              nc.gpsimd.dma_start(W2_s[:], W2.ap().rearrange("(c p) k -> p c k", c=4))
              W3_s = cp.tile([128, 2], F16)
              nc.gpsimd.dma_start(W3_s[:], W3.ap().rearrange("(c p) o -> p (c o)", c=2))
              b2_s = cp.tile([128, 2], F32)
              nc.sync.dma_start(b2_s[:], b2.ap().rearrange("(c p) -> p c", c=2))
              b3_s = cp.tile([1, 1], F32)
              nc.sync.dma_start(b3_s[:], b3.ap().unsqueeze(0))
              # pair-major logits: partition p holds rows 2p,2p+1 so the
              # stage-D strip activation writes it directly (no SBUF->SBUF
              # DMA); the output DMA un-reshapes via the DRAM-side AP
              logits_sb = cp.tile([npair, 2 * nj], F32)

              # tpfull: 3-deep manual stationary ring whose last partition
              # row permanently holds b1 (the K=33 bias trick lives in the
              # stationary, so it is filled only once, not per pair)
              TPB = 3
              tpfull = cp.tile([D + 1, TPB, 2, H], F16)
              for t in range(TPB):
                  nc.gpsimd.dma_start(
                      tpfull[D : D + 1, t, :, :],
                      b1.ap().unsqueeze(0).broadcast_to([2, H]).unsqueeze(0),
                  )

              # ---------- stage A: tmp2xT[c, i, :] ----------
              sbA = None
              for n in range(D):
                  psA = ps.tile([rows, H], F32, tag="ac", bufs=4)
                  nc.tensor.matmul(psA[:], ziT_s[:], W1_s[:, n, :], start=True, stop=True)
                  if n % 2 == 0:
                      sbA = wp.tile([rows, 2, H], F16, tag="sa", bufs=6)
                      nc.vector.tensor_copy(sbA[:, 0, :], psA[:])
                  else:
                      nc.scalar.copy(sbA[:, 1, :], psA[:])
                      # SP/HWDGE; transpose to c-major on the DRAM-side AP
                      nc.sync.dma_start(
                          tmp2xT.ap()[n - 1 : n + 1, :, :].rearrange("c i h -> i c h"),
                          sbA[:],
                      )

              # ---------- main loop over i-pairs (software-pipelined) ----------
              # Stage C/D of pair p is emitted after stage B of pair p+1 so the
              # PE never waits on the DVE relu-eviction of h1T.
              def stage_B(p):
                  s = p % TPB
                  # straight strided read from the c-major scratch
                  nc.sync.dma_start(
                      tpfull[0:D, s, :, :], tmp2xT.ap()[:, 2 * p : 2 * p + 2, :]
                  )
                  tp = tpfull[:, s]
                  h1T = wp.tile([128, 4, 2 * nj], F16, tag="h1")
                  for i in range(2):
                      for hh in range(2):  # psB split into 1-bank halves
                          psB = ps.tile([128, 2, nj], F32, tag="b")
                          for hx in range(2):
                              hc = 2 * hh + hx
                              nc.tensor.matmul(
                                  psB[:, hx, :],
                                  tp[:, i, hc * 128 : (hc + 1) * 128],
                                  zTx_s[:],
                                  start=(hx == 0),
                                  stop=(hx == 1),
                              )
                          # relu; bias already folded in via the K=33 ones row
                          nc.vector.tensor_scalar(
                              h1T[:, 2 * hh : 2 * hh + 2, i * nj : (i + 1) * nj],
                              psB[:], 0.0, None, ALU.max,
                          )
                  return h1T

              def stage_CD(p, h1T):
                  h2T = wp.tile([128, 2, 2 * nj], F16, tag="h2")
                  for kc in range(2):
                      psC = ps.tile([128, 2 * nj], F32, tag="ac", bufs=4)
                      for hc in range(4):
                          nc.tensor.matmul(
                              psC[:],
                              W2_s[:, hc, kc * 128 : (kc + 1) * 128],
                              h1T[:, hc, :],
                              start=(hc == 0),
                              stop=(hc == 3),
                          )
                      nc.scalar.activation(
                          h2T[:, kc, :], psC[:], AF.Relu, bias=b2_s[:, kc : kc + 1]
                      )
                  psD = ps.tile([1, 2 * nj], F32, tag="d")
                  nc.tensor.matmul(psD[:], W3_s[:, 0:1], h2T[:, 0, :], start=True, stop=False)
                  nc.tensor.matmul(psD[:], W3_s[:, 1:2], h2T[:, 1, :], start=False, stop=True)
                  # engines cannot write at partition base p, only DMA can
                  # shift partitions; Pool's software DGE is idle so the tiny
                  # copy is free
                  strip = wp.tile([1, 2 * nj], F32, tag="st")
                  nc.scalar.activation(strip[:], psD[:], AF.Identity, bias=b3_s[:])
                  nc.gpsimd.dma_start(logits_sb[p : p + 1, :], strip[:])

              # epilogue split at pair 32: engine partition bases must be
              # 0/32/64/96
              cmap_sb = cp.tile([npair, 2 * nj], F32)
              half_pairs = min(32, npair)
              logits_ov = logits_o.ap().rearrange("(p i) j -> p (i j)", i=2)
              cmap_ov = cmap_o.ap().rearrange("(p i) j -> p (i j)", i=2)

              def epilogue_half(h):
                  psl = slice(0, half_pairs) if h == 0 else slice(half_pairs, npair)
                  if psl.start >= psl.stop:
                      return
                  # logits dep is ready when emitted; cmap DMA goes on the Act
                  # queue right behind its sigmoid so SP's tp triggers for the
                  # second half are never blocked waiting on it
                  nc.sync.dma_start(logits_ov[psl, :], logits_sb[psl, :])
                  nc.scalar.activation(cmap_sb[psl, :], logits_sb[psl, :], AF.Sigmoid)
                  nc.scalar.dma_start(cmap_ov[psl, :], cmap_sb[psl, :])

              prev = None
              for p in range(npair):
                  h1T_p = stage_B(p)
                  if prev is not None:
                      stage_CD(*prev)
                      if prev[0] == half_pairs - 1:
                          epilogue_half(0)
                  prev = (p, h1T_p)
              stage_CD(*prev)
              epilogue_half(1)

    nc.compile()
    return nc


def _active_idx(motif_mask):
    return [np.nonzero(np.asarray(motif_mask[b]) > 0.5)[0] for b in range(B)]


def _core_rows(idx_b, half):
    n = len(idx_b)
    h = (n + 1) // 2
    return idx_b[:h] if half == 0 else idx_b[h:]


def _in_maps(z, motif_mask, W1, b1, W2, b2, W3, b3, rows=ROWS, nj=NJ):
    z = np.ascontiguousarray(np.asarray(z, dtype=np.float32))
    motif_mask = np.asarray(motif_mask, dtype=np.float32)
    W1 = np.ascontiguousarray(np.asarray(W1, dtype=np.float32)).reshape(D * D, H)
    W2 = np.ascontiguousarray(np.asarray(W2, dtype=np.float32)).reshape(H, H // 2)
    W3 = np.ascontiguousarray(np.asarray(W3, dtype=np.float32)).reshape(H // 2, 1)
    b1 = np.ascontiguousarray(np.asarray(b1, dtype=np.float32)).reshape(H)
    b2 = np.ascontiguousarray(np.asarray(b2, dtype=np.float32)).reshape(H // 2)
    b3 = np.ascontiguousarray(np.asarray(b3, dtype=np.float32)).reshape(1)
    idx = _active_idx(motif_mask)
    maps = []
    for c in range(NCORES):
        b, half = divmod(c, 2)
        act = idx[b]
        n = len(act)
        rows_c = _core_rows(act, half)
        zb = z[b]  # (N, D)
        ziT = np.zeros((D, rows), np.float32)
        ziT[:, : len(rows_c)] = zb[rows_c].T
        zTx = np.zeros((D + 1, nj), np.float32)
        zTx[:D, :n] = zb[act].T
        zTx[D, :n] = 1.0
        maps.append(
            {
                "ziT": ziT,
                "zTx": zTx,
                "W1": W1,
                "W2": W2,
                "W3": W3,
                "b1": b1,
                "b2": b2,
                "b3": b3,
            }
        )
    return maps


def kernel(z, motif_mask, residue_mask, W1, b1, W2, b2, W3, b3):
    global _cached_nc
    motif_np = np.asarray(motif_mask, dtype=np.float32)
    idx = _active_idx(motif_np)
    max_n = max(len(ix) for ix in idx)

    if max_n <= NJ:
        rows, nj, key = ROWS, NJ, 1
    else:
        rows, nj, key = 128, 256, "full"  # universal fallback, any mask fits
    if key not in _cached_nc:
        _cached_nc[key] = _build(rows=rows, nj=nj)
    nc = _cached_nc[key]

    maps = _in_maps(z, motif_mask, W1, b1, W2, b2, W3, b3, rows=rows, nj=nj)
    res = run_bass_kernel_spmd(nc, maps, list(range(NCORES)))

    # masked entries are exact: logit = 0, cmap = sigmoid(0) = 0.5
    logits = np.zeros((B, N, N), np.float32)
    cmap = np.full((B, N, N), 0.5, np.float32)
    for c in range(NCORES):
        b, half = divmod(c, 2)
        act = idx[b]
        n = len(act)
        rows_c = _core_rows(act, half)
        if len(rows_c) == 0:
            continue
        lg = res.results[c]["logits"][: len(rows_c), :n]
        cm = res.results[c]["cmap"][: len(rows_c), :n]
        logits[b][np.ix_(rows_c, act)] = lg
        cmap[b][np.ix_(rows_c, act)] = cm
    return cmap, logits


# revision 24
# speedup vs baseline: 1.7074x; 1.6066x over previous
"""Trainium2 Bass kernel for the pairwise contact-map decoder.

Reference computation (per batch b):
    tmp[b,i,c,h] = sum_a z[b,i,a] * W1[(a,c),h]
    h1[b,i,j,h]  = relu(sum_c tmp[b,i,c,h] * z[b,j,c] + b1[h])
    h2[b,i,j,k]  = relu(sum_h h1[b,i,j,h] * W2[h,k] + b2[k])
    logit[b,i,j] = (sum_k h2[b,i,j,k] * W3[k,0] + b3) * motif[b,i] * motif[b,j]
    cmap         = sigmoid(logit)

Key structural fact: the outer motif mask zeroes every (i,j) where either
index is masked out, making logit exactly 0 and cmap exactly sigmoid(0)=0.5
there.  So only the active submatrix (rows/cols with motif==1, ~50% each,
~25% of the grid) ever needs computing.  The host gathers active rows/cols,
the device computes the dense active block, and the host scatters results
into a 0 / 0.5 - filled full-size output.  Exact: masked entries match the
reference bit-for-bit, active entries follow the same fp16 path as before.

Sharding: 8 cores, core = 2*b + half; the two cores of a batch split its
active rows.  Compiled slab: ROWS=72 i-rows x NJ=144 j-cols (covers any
mask with <=144 active per batch; a full-size 128x256 variant is built
lazily as fallback for larger masks).

On-core dataflow (identical pipeline to the full-grid version):
  stage A (float32r matmuls): tmp2[i, c, h] staged to an fp16 DRAM scratch
           with an extra c-row holding b1 (bias folded via K=33).
  per i-pair: stage B  h1T[h,(i,j)] = tmp2_i.T @ zTx  (K=33 includes bias)
              stage C  h2T[k,(i,j)] accumulate over 4 h-chunks of W2
              stage D  logits strip (1, 2*NJ) via W3 chunks
  Stage C/D of pair p runs after stage B of pair p+1 (software pipeline).
  epilogue: sigmoid + DMA out in row-halves (no mask work on device).
"""

import numpy as np

import concourse.bass as bass
import concourse.mybir as mybir
import concourse.tile as tile
from concourse import bacc
from concourse.bass_utils import run_bass_kernel_spmd

B, N, D, H = 4, 256, 32, 512
DT = mybir.dt
F32, F32R, F16 = DT.float32, DT.float32r, DT.float16
AF = mybir.ActivationFunctionType
ALU = mybir.AluOpType
NCORES = 8
ROWS = 72   # padded active i-rows per core
NJ = 144    # padded active j-cols per batch

_cached_nc = {}


from contextlib import nullcontext as _nullcontext


def _r(ap):
    return ap.bitcast(F32R)


def _build(reps=1, rows=ROWS, nj=NJ, unroll=False):
    npair = rows // 2
    nc = bacc.Bacc("TRN2", target_bir_lowering=False, debug=False, num_devices=NCORES)

    ziT = nc.dram_tensor("ziT", [D, rows], F32, kind="ExternalInput")
    zTx = nc.dram_tensor("zTx", [D + 1, nj], F32, kind="ExternalInput")
    W1 = nc.dram_tensor("W1", [D * D, H], F32, kind="ExternalInput")
    W2 = nc.dram_tensor("W2", [H, H // 2], F32, kind="ExternalInput")
    W3 = nc.dram_tensor("W3", [H // 2, 1], F32, kind="ExternalInput")
    b1 = nc.dram_tensor("b1", [H], F32, kind="ExternalInput")
    b2 = nc.dram_tensor("b2", [H // 2], F32, kind="ExternalInput")
    b3 = nc.dram_tensor("b3", [1], F32, kind="ExternalInput")
    logits_o = nc.dram_tensor("logits", [rows, nj], F32, kind="ExternalOutput")
    cmap_o = nc.dram_tensor("cmap", [rows, nj], F32, kind="ExternalOutput")
    # c-major scratch: transpose lives on the DRAM-side write AP, reads are
    # clean strided loads (SBUF APs cannot cross partitions)
    tmp2xT = nc.dram_tensor("tmp2xT", [D, rows, H], F16)

    with tile.TileContext(nc) as tc:
        with (
            tc.tile_pool(name="const", bufs=1) as cp,
            tc.tile_pool(name="work", bufs=3) as wp,
            tc.tile_pool(name="ps", bufs=2, space="PSUM") as ps,
        ):
          rep_ctx = (
              tc.For_i(0, reps, 1) if reps > 1 and not unroll else _nullcontext()
          )
          for _rep in range(reps if unroll else 1):
           with rep_ctx if _rep == 0 else _nullcontext():
              # ---------- persistent loads ----------
              ziT_s = cp.tile([D, rows], F32R)
              nc.sync.dma_start(ziT_s[:], _r(ziT.ap()))
              W1v = _r(W1.ap().rearrange("(a c) h -> a c h", a=D))
              W1_s = cp.tile([D, D, H], F32R)
              nc.sync.dma_start(W1_s[:, 0:4, :], W1v[:, 0:4, :])
              nc.sync.dma_start(W1_s[:, 4:8, :], W1v[:, 4:8, :])
              for q in range(1, 4):
                  nc.sync.dma_start(W1_s[:, 8 * q : 8 * (q + 1), :], W1v[:, 8 * q : 8 * (q + 1), :])
              zTx_s = cp.tile([D + 1, nj], F16)
              nc.gpsimd.dma_start(zTx_s[:], zTx.ap())
              W2_s = cp.tile([128, 4, 256], F16)
              nc.gpsimd.dma_start(W2_s[:], W2.ap().rearrange("(c p) k -> p c k", c=4))
              W3_s = cp.tile([128, 2], F16)
              nc.gpsimd.dma_start(W3_s[:], W3.ap().rearrange("(c p) o -> p (c o)", c=2))
              b2_s = cp.tile([128, 2], F32)
              nc.sync.dma_start(b2_s[:], b2.ap().rearrange("(c p) -> p c", c=2))
              b3_s = cp.tile([1, 1], F32)
              nc.sync.dma_start(b3_s[:], b3.ap().unsqueeze(0))
              # pair-major logits: partition p holds rows 2p,2p+1 so the
              # stage-D strip activation writes it directly (no SBUF->SBUF
              # DMA); the output DMA un-reshapes via the DRAM-side AP
              logits_sb = cp.tile([npair, 2 * nj], F32)

              # tpfull: 3-deep manual stationary ring whose last partition
              # row permanently holds b1 (the K=33 bias trick lives in the
              # stationary, so it is filled only once, not per pair)
              TPB = 4
              tpfull = cp.tile([D + 1, TPB, 2, H], F16)
              for t in range(TPB):
                  nc.gpsimd.dma_start(
                      tpfull[D : D + 1, t, :, :],
                      b1.ap().unsqueeze(0).broadcast_to([2, H]).unsqueeze(0),
                  )

              # ---------- stage A: tmp2xT[c, i, :] ----------
              sbA = None
              for n in range(D):
                  psA = ps.tile([rows, H], F32, tag="ac", bufs=3)
                  nc.tensor.matmul(psA[:], ziT_s[:], W1_s[:, n, :], start=True, stop=True)
                  if n % 2 == 0:
                      sbA = wp.tile([rows, 2, H], F16, tag="sa", bufs=6)
                      nc.vector.tensor_copy(sbA[:, 0, :], psA[:])
                  else:
                      nc.scalar.copy(sbA[:, 1, :], psA[:])
                      # SP/HWDGE; transpose to c-major on the DRAM-side AP
                      nc.sync.dma_start(
                          tmp2xT.ap()[n - 1 : n + 1, :, :].rearrange("c i h -> i c h"),
                          sbA[:],
                      )

              # ---------- main loop over i-pairs (software-pipelined) ----------
              # 3-stage pipeline: B(p), C(p-1), D(p-2).  Each cross-engine
              # handoff (PE->DVE/Act evict -> PE) gets a full pair of slack;
              # HW semaphore latency is far larger than the cost model's
              # 100ns, so depth, not speed, is what hides it.
              def tp_fetch(p):
                  if p < npair:
                      nc.sync.dma_start(
                          tpfull[0:D, p % TPB, :, :],
                          tmp2xT.ap()[:, 2 * p : 2 * p + 2, :],
                      )

              def stage_B(p):
                  tp_fetch(p + 1)  # prefetch next pair's stationary
                  tp = tpfull[:, p % TPB]
                  h1T = wp.tile([128, 4, 2 * nj], F16, tag="h1", bufs=5)
                  for i in range(2):
                      for hh in range(2):  # psB split into 1-bank halves
                          psB = ps.tile([128, 2, nj], F32, tag="b", bufs=4)
                          for hx in range(2):
                              hc = 2 * hh + hx
                              nc.tensor.matmul(
                                  psB[:, hx, :],
                                  tp[:, i, hc * 128 : (hc + 1) * 128],
                                  zTx_s[:],
                                  start=(hx == 0),
                                  stop=(hx == 1),
                              )
                          # relu; bias already folded in via the K=33 ones row
                          # (3 evicts on DVE, 1 on Act to balance load)
                          dst = h1T[:, 2 * hh : 2 * hh + 2, i * nj : (i + 1) * nj]
                          if i == 1 and hh == 1:
                              nc.scalar.activation(dst, psB[:], AF.Relu)
                          else:
                              nc.vector.tensor_scalar(dst, psB[:], 0.0, None, ALU.max)
                  return h1T

              def stage_C(p, h1T):
                  h2T = wp.tile([128, 2, 2 * nj], F16, tag="h2", bufs=5)
                  for kc in range(2):
                      psC = ps.tile([128, 2 * nj], F32, tag="ac", bufs=3)
                      for hc in range(4):
                          nc.tensor.matmul(
                              psC[:],
                              W2_s[:, hc, kc * 128 : (kc + 1) * 128],
                              h1T[:, hc, :],
                              start=(hc == 0),
                              stop=(hc == 3),
                          )
                      nc.scalar.activation(
                          h2T[:, kc, :], psC[:], AF.Relu, bias=b2_s[:, kc : kc + 1]
                      )
                  return h2T

              def stage_D(p, h2T):
                  psD = ps.tile([1, 2 * nj], F32, tag="d", bufs=1)
                  nc.tensor.matmul(psD[:], W3_s[:, 0:1], h2T[:, 0, :], start=True, stop=False)
                  nc.tensor.matmul(psD[:], W3_s[:, 1:2], h2T[:, 1, :], start=False, stop=True)
                  # engines cannot write at partition base p, only DMA can
                  # shift partitions; Pool's software DGE is idle so the tiny
                  # copy is free
                  strip = wp.tile([1, 2 * nj], F32, tag="st")
                  nc.scalar.activation(strip[:], psD[:], AF.Identity, bias=b3_s[:])
                  nc.gpsimd.dma_start(logits_sb[p : p + 1, :], strip[:])

              # epilogue split at pair 32: engine partition bases must be
              # 0/32/64/96
              cmap_sb = cp.tile([npair, 2 * nj], F32)
              half_pairs = min(32, npair)
              logits_ov = logits_o.ap().rearrange("(p i) j -> p (i j)", i=2)
              cmap_ov = cmap_o.ap().rearrange("(p i) j -> p (i j)", i=2)

              def epilogue_half(h):
                  psl = slice(0, half_pairs) if h == 0 else slice(half_pairs, npair)
                  if psl.start >= psl.stop:
                      return
                  # logits dep is ready when emitted; cmap DMA goes on the Act
                  # queue right behind its sigmoid so SP's tp triggers for the
                  # second half are never blocked waiting on it
                  nc.sync.dma_start(logits_ov[psl, :], logits_sb[psl, :])
                  nc.scalar.activation(cmap_sb[psl, :], logits_sb[psl, :], AF.Sigmoid)
                  nc.scalar.dma_start(cmap_ov[psl, :], cmap_sb[psl, :])

              tp_fetch(0)
              h1q, h2q = [], []

              def drain_c():
                  q, h1 = h1q.pop(0)
                  h2q.append((q, stage_C(q, h1)))

              def drain_d():
                  q, h2 = h2q.pop(0)
                  stage_D(q, h2)
                  if q == half_pairs - 1:
                      epilogue_half(0)

              for p in range(npair):
                  h1q.append((p, stage_B(p)))
                  if len(h1q) > 2:
                      drain_c()
                  if len(h2q) > 2:
                      drain_d()
              while h1q:
                  drain_c()
              while h2q:
                  drain_d()
              epilogue_half(1)

    nc.compile()
    return nc


def _active_idx(motif_mask):
    return [np.nonzero(np.asarray(motif_mask[b]) > 0.5)[0] for b in range(B)]


def _core_rows(idx_b, half):
    n = len(idx_b)
    h = (n + 1) // 2
    return idx_b[:h] if half == 0 else idx_b[h:]


def _in_maps(z, motif_mask, W1, b1, W2, b2, W3, b3, rows=ROWS, nj=NJ):
    z = np.ascontiguousarray(np.asarray(z, dtype=np.float32))
    motif_mask = np.asarray(motif_mask, dtype=np.float32)
    W1 = np.ascontiguousarray(np.asarray(W1, dtype=np.float32)).reshape(D * D, H)
    W2 = np.ascontiguousarray(np.asarray(W2, dtype=np.float32)).reshape(H, H // 2)
    W3 = np.ascontiguousarray(np.asarray(W3, dtype=np.float32)).reshape(H // 2, 1)
    b1 = np.ascontiguousarray(np.asarray(b1, dtype=np.float32)).reshape(H)
    b2 = np.ascontiguousarray(np.asarray(b2, dtype=np.float32)).reshape(H // 2)
    b3 = np.ascontiguousarray(np.asarray(b3, dtype=np.float32)).reshape(1)
    idx = _active_idx(motif_mask)
    maps = []
    for c in range(NCORES):
        b, half = divmod(c, 2)
        act = idx[b]
        n = len(act)
        rows_c = _core_rows(act, half)
        zb = z[b]  # (N, D)
        ziT = np.zeros((D, rows), np.float32)
        ziT[:, : len(rows_c)] = zb[rows_c].T
        zTx = np.zeros((D + 1, nj), np.float32)
        zTx[:D, :n] = zb[act].T
        zTx[D, :n] = 1.0
        maps.append(
            {
                "ziT": ziT,
                "zTx": zTx,
                "W1": W1,
                "W2": W2,
                "W3": W3,
                "b1": b1,
                "b2": b2,
                "b3": b3,
            }
        )
    return maps


def kernel(z, motif_mask, residue_mask, W1, b1, W2, b2, W3, b3):
    global _cached_nc
    motif_np = np.asarray(motif_mask, dtype=np.float32)
    idx = _active_idx(motif_np)
    max_n = max(len(ix) for ix in idx)

    if max_n <= NJ:
        rows, nj, key = ROWS, NJ, 1
    else:
        rows, nj, key = 128, 256, "full"  # universal fallback, any mask fits
    if key not in _cached_nc:
        _cached_nc[key] = _build(rows=rows, nj=nj)
    nc = _cached_nc[key]

    maps = _in_maps(z, motif_mask, W1, b1, W2, b2, W3, b3, rows=rows, nj=nj)
    res = run_bass_kernel_spmd(nc, maps, list(range(NCORES)))

    # masked entries are exact: logit = 0, cmap = sigmoid(0) = 0.5
    logits = np.zeros((B, N, N), np.float32)
    cmap = np.full((B, N, N), 0.5, np.float32)
    for c in range(NCORES):
        b, half = divmod(c, 2)
        act = idx[b]
        n = len(act)
        rows_c = _core_rows(act, half)
        if len(rows_c) == 0:
            continue
        lg = res.results[c]["logits"][: len(rows_c), :n]
        cm = res.results[c]["cmap"][: len(rows_c), :n]
        logits[b][np.ix_(rows_c, act)] = lg
        cmap[b][np.ix_(rows_c, act)] = cm
    return cmap, logits


# revision 28
# speedup vs baseline: 1.7161x; 1.0051x over previous
"""Trainium2 Bass kernel for the pairwise contact-map decoder.

Reference computation (per batch b):
    tmp[b,i,c,h] = sum_a z[b,i,a] * W1[(a,c),h]
    h1[b,i,j,h]  = relu(sum_c tmp[b,i,c,h] * z[b,j,c] + b1[h])
    h2[b,i,j,k]  = relu(sum_h h1[b,i,j,h] * W2[h,k] + b2[k])
    logit[b,i,j] = (sum_k h2[b,i,j,k] * W3[k,0] + b3) * motif[b,i] * motif[b,j]
    cmap         = sigmoid(logit)

Key structural fact: the outer motif mask zeroes every (i,j) where either
index is masked out, making logit exactly 0 and cmap exactly sigmoid(0)=0.5
there.  So only the active submatrix (rows/cols with motif==1, ~50% each,
~25% of the grid) ever needs computing.  The host gathers active rows/cols,
the device computes the dense active block, and the host scatters results
into a 0 / 0.5 - filled full-size output.  Exact: masked entries match the
reference bit-for-bit, active entries follow the same fp16 path as before.

Sharding: 8 cores, core = 2*b + half; the two cores of a batch split its
active rows.  Compiled slab: ROWS=72 i-rows x NJ=144 j-cols (covers any
mask with <=144 active per batch; a full-size 128x256 variant is built
lazily as fallback for larger masks).

On-core dataflow (identical pipeline to the full-grid version):
  stage A (float32r matmuls): tmp2[i, c, h] staged to an fp16 DRAM scratch
           with an extra c-row holding b1 (bias folded via K=33).
  per i-pair: stage B  h1T[h,(i,j)] = tmp2_i.T @ zTx  (K=33 includes bias)
              stage C  h2T[k,(i,j)] accumulate over 4 h-chunks of W2
              stage D  logits strip (1, 2*NJ) via W3 chunks
  Stage C/D of pair p runs after stage B of pair p+1 (software pipeline).
  epilogue: sigmoid + DMA out in row-halves (no mask work on device).
"""

import numpy as np

import concourse.bass as bass
import concourse.mybir as mybir
import concourse.tile as tile
from concourse import bacc
from concourse.bass_utils import run_bass_kernel_spmd

B, N, D, H = 4, 256, 32, 512
DT = mybir.dt
F32, F32R, F16 = DT.float32, DT.float32r, DT.float16
AF = mybir.ActivationFunctionType
ALU = mybir.AluOpType
NCORES = 8
ROWS = 72   # padded active i-rows per core
NJ = 144    # padded active j-cols per batch

_cached_nc = {}


from contextlib import nullcontext as _nullcontext


def _r(ap):
    return ap.bitcast(F32R)


def _build(reps=1, rows=ROWS, nj=NJ, unroll=False):
    npair = rows // 2
    nc = bacc.Bacc("TRN2", target_bir_lowering=False, debug=False, num_devices=NCORES)

    ziT = nc.dram_tensor("ziT", [D, rows], F16, kind="ExternalInput")
    zTx = nc.dram_tensor("zTx", [D + 1, nj], F32, kind="ExternalInput")
    W1 = nc.dram_tensor("W1", [D * D, H], F16, kind="ExternalInput")
    W2 = nc.dram_tensor("W2", [H, H // 2], F32, kind="ExternalInput")
    W3 = nc.dram_tensor("W3", [H // 2, 1], F32, kind="ExternalInput")
    b1 = nc.dram_tensor("b1", [H], F32, kind="ExternalInput")
    b2 = nc.dram_tensor("b2", [H // 2], F32, kind="ExternalInput")
    b3 = nc.dram_tensor("b3", [1], F32, kind="ExternalInput")
    logits_o = nc.dram_tensor("logits", [rows, nj], F32, kind="ExternalOutput")
    cmap_o = nc.dram_tensor("cmap", [rows, nj], F32, kind="ExternalOutput")
    # c-major scratch: transpose lives on the DRAM-side write AP, reads are
    # clean strided loads (SBUF APs cannot cross partitions)
    tmp2xT = nc.dram_tensor("tmp2xT", [D, rows, H], F16)

    with tile.TileContext(nc) as tc:
        with (
            tc.tile_pool(name="const", bufs=1) as cp,
            tc.tile_pool(name="work", bufs=3) as wp,
            tc.tile_pool(name="ps", bufs=2, space="PSUM") as ps,
        ):
          rep_ctx = (
              tc.For_i(0, reps, 1) if reps > 1 and not unroll else _nullcontext()
          )
          for _rep in range(reps if unroll else 1):
           with rep_ctx if _rep == 0 else _nullcontext():
              # ---------- persistent loads ----------
              ziT_s = cp.tile([D, rows], F16)
              nc.sync.dma_start(ziT_s[:], ziT.ap())
              W1v = W1.ap().rearrange("(a c) h -> a c h", a=D)
              W1_s = cp.tile([D, D, H], F16)
              nc.sync.dma_start(W1_s[:, 0:4, :], W1v[:, 0:4, :])
              nc.sync.dma_start(W1_s[:, 4:8, :], W1v[:, 4:8, :])
              for q in range(1, 4):
                  nc.sync.dma_start(W1_s[:, 8 * q : 8 * (q + 1), :], W1v[:, 8 * q : 8 * (q + 1), :])
              zTx_s = cp.tile([D + 1, nj], F16)
              nc.gpsimd.dma_start(zTx_s[:], zTx.ap())
              W2_s = cp.tile([128, 4, 256], F16)
              nc.gpsimd.dma_start(W2_s[:], W2.ap().rearrange("(c p) k -> p c k", c=4))
              W3_s = cp.tile([128, 2], F16)
              nc.gpsimd.dma_start(W3_s[:], W3.ap().rearrange("(c p) o -> p (c o)", c=2))
              b2_s = cp.tile([128, 2], F32)
              nc.sync.dma_start(b2_s[:], b2.ap().rearrange("(c p) -> p c", c=2))
              b3_s = cp.tile([1, 1], F32)
              nc.sync.dma_start(b3_s[:], b3.ap().unsqueeze(0))
              # pair-major logits: partition p holds rows 2p,2p+1 so the
              # stage-D strip activation writes it directly (no SBUF->SBUF
              # DMA); the output DMA un-reshapes via the DRAM-side AP
              logits_sb = cp.tile([npair, 2 * nj], F32)

              # tpfull: 3-deep manual stationary ring whose last partition
              # row permanently holds b1 (the K=33 bias trick lives in the
              # stationary, so it is filled only once, not per pair)
              TPB = 4
              tpfull = cp.tile([D + 1, TPB, 2, H], F16)
              for t in range(TPB):
                  nc.gpsimd.dma_start(
                      tpfull[D : D + 1, t, :, :],
                      b1.ap().unsqueeze(0).broadcast_to([2, H]).unsqueeze(0),
                  )

              # ---------- stage A: tmp2xT[c, i, :] ----------
              sbA = None
              for n in range(D):
                  psA = ps.tile([rows, H], F32, tag="ac", bufs=3)
                  nc.tensor.matmul(psA[:], ziT_s[:], W1_s[:, n, :], start=True, stop=True)
                  if n % 2 == 0:
                      sbA = wp.tile([rows, 2, H], F16, tag="sa", bufs=6)
                      nc.vector.tensor_copy(sbA[:, 0, :], psA[:])
                  else:
                      nc.scalar.copy(sbA[:, 1, :], psA[:])
                      # SP/HWDGE; transpose to c-major on the DRAM-side AP
                      nc.sync.dma_start(
                          tmp2xT.ap()[n - 1 : n + 1, :, :].rearrange("c i h -> i c h"),
                          sbA[:],
                      )

              # ---------- main loop over i-pairs (software-pipelined) ----------
              # 3-stage pipeline: B(p), C(p-1), D(p-2).  Each cross-engine
              # handoff (PE->DVE/Act evict -> PE) gets a full pair of slack;
              # HW semaphore latency is far larger than the cost model's
              # 100ns, so depth, not speed, is what hides it.
              def tp_fetch(p):
                  if p < npair:
                      nc.sync.dma_start(
                          tpfull[0:D, p % TPB, :, :],
                          tmp2xT.ap()[:, 2 * p : 2 * p + 2, :],
                      )

              def stage_B(p):
                  tp_fetch(p + 2)  # prefetch two pairs ahead
                  tp = tpfull[:, p % TPB]
                  h1T = wp.tile([128, 4, 2 * nj], F16, tag="h1", bufs=5)
                  for i in range(2):
                      # quarters padded to 256 so each matmul output stays
                      # inside one 2KB PSUM bank (accumulation is per-bank)
                      psB = ps.tile([128, 4, 256], F32, tag="b", bufs=2)
                      for hc in range(4):
                          nc.tensor.matmul(
                              psB[:, hc, 0:nj],
                              tp[:, i, hc * 128 : (hc + 1) * 128],
                              zTx_s[:],
                              start=(hc % 2 == 0),
                              stop=(hc % 2 == 1),
                          )
                      # relu; bias already folded in via the K=33 ones row
                      nc.vector.tensor_scalar(
                          h1T[:, :, i * nj : (i + 1) * nj], psB[:, :, 0:nj],
                          0.0, None, ALU.max,
                      )
                  return h1T

              def stage_C(p, h1T):
                  h2T = wp.tile([128, 2, 2 * nj], F16, tag="h2", bufs=5)
                  for kc in range(2):
                      psC = ps.tile([128, 2 * nj], F32, tag="ac", bufs=3)
                      for hc in range(4):
                          nc.tensor.matmul(
                              psC[:],
                              W2_s[:, hc, kc * 128 : (kc + 1) * 128],
                              h1T[:, hc, :],
                              start=(hc == 0),
                              stop=(hc == 3),
                          )
                      nc.scalar.activation(
                          h2T[:, kc, :], psC[:], AF.Relu, bias=b2_s[:, kc : kc + 1]
                      )
                  return h2T

              def stage_D(p, h2T):
                  psD = ps.tile([1, 2 * nj], F32, tag="d", bufs=1)
                  nc.tensor.matmul(psD[:], W3_s[:, 0:1], h2T[:, 0, :], start=True, stop=False)
                  nc.tensor.matmul(psD[:], W3_s[:, 1:2], h2T[:, 1, :], start=False, stop=True)
                  # engines cannot write at partition base p, only DMA can
                  # shift partitions; Pool's software DGE is idle so the tiny
                  # copy is free
                  strip = wp.tile([1, 2 * nj], F32, tag="st")
                  nc.scalar.activation(strip[:], psD[:], AF.Identity, bias=b3_s[:])
                  nc.gpsimd.dma_start(logits_sb[p : p + 1, :], strip[:])

              # epilogue split at pair 32: engine partition bases must be
              # 0/32/64/96
              cmap_sb = cp.tile([npair, 2 * nj], F32)
              half_pairs = min(32, npair)
              logits_ov = logits_o.ap().rearrange("(p i) j -> p (i j)", i=2)
              cmap_ov = cmap_o.ap().rearrange("(p i) j -> p (i j)", i=2)

              def epilogue_half(h):
                  psl = slice(0, half_pairs) if h == 0 else slice(half_pairs, npair)
                  if psl.start >= psl.stop:
                      return
                  # logits dep is ready when emitted; cmap DMA goes on the Act
                  # queue right behind its sigmoid so SP's tp triggers for the
                  # second half are never blocked waiting on it
                  nc.sync.dma_start(logits_ov[psl, :], logits_sb[psl, :])
                  nc.scalar.activation(cmap_sb[psl, :], logits_sb[psl, :], AF.Sigmoid)
                  nc.scalar.dma_start(cmap_ov[psl, :], cmap_sb[psl, :])

              tp_fetch(0)
              tp_fetch(1)
              h1q, h2q = [], []

              def drain_c():
                  q, h1 = h1q.pop(0)
                  h2q.append((q, stage_C(q, h1)))

              def drain_d():
                  q, h2 = h2q.pop(0)
                  stage_D(q, h2)
                  if q == half_pairs - 1:
                      epilogue_half(0)

              for p in range(npair):
                  h1q.append((p, stage_B(p)))
                  if len(h1q) > 2:
                      drain_c()
                  if len(h2q) > 2:
                      drain_d()
              while h1q:
                  drain_c()
              while h2q:
                  drain_d()
              epilogue_half(1)

    nc.compile()
    return nc


def _active_idx(motif_mask):
    return [np.nonzero(np.asarray(motif_mask[b]) > 0.5)[0] for b in range(B)]


def _core_rows(idx_b, half):
    n = len(idx_b)
    h = (n + 1) // 2
    return idx_b[:h] if half == 0 else idx_b[h:]


def _in_maps(z, motif_mask, W1, b1, W2, b2, W3, b3, rows=ROWS, nj=NJ):
    z = np.ascontiguousarray(np.asarray(z, dtype=np.float32))
    motif_mask = np.asarray(motif_mask, dtype=np.float32)
    W1 = np.ascontiguousarray(np.asarray(W1, dtype=np.float16)).reshape(D * D, H)
    W2 = np.ascontiguousarray(np.asarray(W2, dtype=np.float32)).reshape(H, H // 2)
    W3 = np.ascontiguousarray(np.asarray(W3, dtype=np.float32)).reshape(H // 2, 1)
    b1 = np.ascontiguousarray(np.asarray(b1, dtype=np.float32)).reshape(H)
    b2 = np.ascontiguousarray(np.asarray(b2, dtype=np.float32)).reshape(H // 2)
    b3 = np.ascontiguousarray(np.asarray(b3, dtype=np.float32)).reshape(1)
    idx = _active_idx(motif_mask)
    maps = []
    for c in range(NCORES):
        b, half = divmod(c, 2)
        act = idx[b]
        n = len(act)
        rows_c = _core_rows(act, half)
        zb = z[b]  # (N, D)
        ziT = np.zeros((D, rows), np.float16)
        ziT[:, : len(rows_c)] = zb[rows_c].T.astype(np.float16)
        zTx = np.zeros((D + 1, nj), np.float32)
        zTx[:D, :n] = zb[act].T
        zTx[D, :n] = 1.0
        maps.append(
            {
                "ziT": ziT,
                "zTx": zTx,
                "W1": W1,
                "W2": W2,
                "W3": W3,
                "b1": b1,
                "b2": b2,
                "b3": b3,
            }
        )
    return maps


def kernel(z, motif_mask, residue_mask, W1, b1, W2, b2, W3, b3):
    global _cached_nc
    motif_np = np.asarray(motif_mask, dtype=np.float32)
    idx = _active_idx(motif_np)
    max_n = max(len(ix) for ix in idx)

    if max_n <= NJ:
        rows, nj, key = ROWS, NJ, 1
    else:
        rows, nj, key = 128, 256, "full"  # universal fallback, any mask fits
    if key not in _cached_nc:
        _cached_nc[key] = _build(rows=rows, nj=nj)
    nc = _cached_nc[key]

    maps = _in_maps(z, motif_mask, W1, b1, W2, b2, W3, b3, rows=rows, nj=nj)
    res = run_bass_kernel_spmd(nc, maps, list(range(NCORES)))

    # masked entries are exact: logit = 0, cmap = sigmoid(0) = 0.5
    logits = np.zeros((B, N, N), np.float32)
    cmap = np.full((B, N, N), 0.5, np.float32)
    for c in range(NCORES):
        b, half = divmod(c, 2)
        act = idx[b]
        n = len(act)
        rows_c = _core_rows(act, half)
        if len(rows_c) == 0:
            continue
        lg = res.results[c]["logits"][: len(rows_c), :n]
        cm = res.results[c]["cmap"][: len(rows_c), :n]
        logits[b][np.ix_(rows_c, act)] = lg
        cmap[b][np.ix_(rows_c, act)] = cm
    return cmap, logits


# revision 29
# speedup vs baseline: 1.8099x; 1.0546x over previous
"""Trainium2 Bass kernel for the pairwise contact-map decoder.

Reference computation (per batch b):
    tmp[b,i,c,h] = sum_a z[b,i,a] * W1[(a,c),h]
    h1[b,i,j,h]  = relu(sum_c tmp[b,i,c,h] * z[b,j,c] + b1[h])
    h2[b,i,j,k]  = relu(sum_h h1[b,i,j,h] * W2[h,k] + b2[k])
    logit[b,i,j] = (sum_k h2[b,i,j,k] * W3[k,0] + b3) * motif[b,i] * motif[b,j]
    cmap         = sigmoid(logit)

Key structural fact: the outer motif mask zeroes every (i,j) where either
index is masked out, making logit exactly 0 and cmap exactly sigmoid(0)=0.5
there.  So only the active submatrix (rows/cols with motif==1, ~50% each,
~25% of the grid) ever needs computing.  The host gathers active rows/cols,
the device computes the dense active block, and the host scatters results
into a 0 / 0.5 - filled full-size output.  Exact: masked entries match the
reference bit-for-bit, active entries follow the same fp16 path as before.

Sharding: 8 cores, core = 2*b + half; the two cores of a batch split its
active rows.  Compiled slab: ROWS=72 i-rows x NJ=144 j-cols (covers any
mask with <=144 active per batch; a full-size 128x256 variant is built
lazily as fallback for larger masks).

On-core dataflow (identical pipeline to the full-grid version):
  stage A (float32r matmuls): tmp2[i, c, h] staged to an fp16 DRAM scratch
           with an extra c-row holding b1 (bias folded via K=33).
  per i-pair: stage B  h1T[h,(i,j)] = tmp2_i.T @ zTx  (K=33 includes bias)
              stage C  h2T[k,(i,j)] accumulate over 4 h-chunks of W2
              stage D  logits strip (1, 2*NJ) via W3 chunks
  Stage C/D of pair p runs after stage B of pair p+1 (software pipeline).
  epilogue: sigmoid + DMA out in row-halves (no mask work on device).
"""

import numpy as np

import concourse.bass as bass
import concourse.mybir as mybir
import concourse.tile as tile
from concourse import bacc
from concourse.bass_utils import run_bass_kernel_spmd

B, N, D, H = 4, 256, 32, 512
DT = mybir.dt
F32, F32R, F16 = DT.float32, DT.float32r, DT.float16
AF = mybir.ActivationFunctionType
ALU = mybir.AluOpType
NCORES = 8
ROWS = 72   # padded active i-rows per core
NJ = 144    # padded active j-cols per batch

_cached_nc = {}


from contextlib import nullcontext as _nullcontext


def _r(ap):
    return ap.bitcast(F32R)


def _build(reps=1, rows=ROWS, nj=NJ, unroll=False):
    npair = rows // 2
    nc = bacc.Bacc("TRN2", target_bir_lowering=False, debug=False, num_devices=NCORES)

    ziT = nc.dram_tensor("ziT", [D, rows], F16, kind="ExternalInput")
    zTx = nc.dram_tensor("zTx", [D + 1, nj], F32, kind="ExternalInput")
    W1 = nc.dram_tensor("W1", [D * D, H], F16, kind="ExternalInput")
    W2 = nc.dram_tensor("W2", [H, H // 2], F32, kind="ExternalInput")
    W3 = nc.dram_tensor("W3", [H // 2, 1], F32, kind="ExternalInput")
    b1 = nc.dram_tensor("b1", [H], F32, kind="ExternalInput")
    b2 = nc.dram_tensor("b2", [H // 2], F32, kind="ExternalInput")
    b3 = nc.dram_tensor("b3", [1], F32, kind="ExternalInput")
    logits_o = nc.dram_tensor("logits", [rows, nj], F32, kind="ExternalOutput")
    cmap_o = nc.dram_tensor("cmap", [rows, nj], F32, kind="ExternalOutput")
    # c-major scratch: transpose lives on the DRAM-side write AP, reads are
    # clean strided loads (SBUF APs cannot cross partitions)
    tmp2xT = nc.dram_tensor("tmp2xT", [D, rows, H], F16)

    with tile.TileContext(nc) as tc:
        with (
            tc.tile_pool(name="const", bufs=1) as cp,
            tc.tile_pool(name="work", bufs=3) as wp,
            tc.tile_pool(name="ps", bufs=2, space="PSUM") as ps,
        ):
          rep_ctx = (
              tc.For_i(0, reps, 1) if reps > 1 and not unroll else _nullcontext()
          )
          for _rep in range(reps if unroll else 1):
           with rep_ctx if _rep == 0 else _nullcontext():
              # ---------- persistent loads ----------
              ziT_s = cp.tile([D, rows], F16)
              nc.sync.dma_start(ziT_s[:], ziT.ap())
              W1v = W1.ap().rearrange("(a c) h -> a c h", a=D)
              W1_s = cp.tile([D, D, H], F16)
              nc.sync.dma_start(W1_s[:, 0:4, :], W1v[:, 0:4, :])
              nc.sync.dma_start(W1_s[:, 4:8, :], W1v[:, 4:8, :])
              for q in range(1, 4):
                  nc.sync.dma_start(W1_s[:, 8 * q : 8 * (q + 1), :], W1v[:, 8 * q : 8 * (q + 1), :])
              zTx_s = cp.tile([D + 1, nj], F16)
              nc.gpsimd.dma_start(zTx_s[:], zTx.ap())
              W2_s = cp.tile([128, 4, 256], F16)
              nc.gpsimd.dma_start(W2_s[:], W2.ap().rearrange("(c p) k -> p c k", c=4))
              W3_s = cp.tile([128, 2], F16)
              nc.gpsimd.dma_start(W3_s[:], W3.ap().rearrange("(c p) o -> p (c o)", c=2))
              b2_s = cp.tile([128, 2], F32)
              nc.sync.dma_start(b2_s[:], b2.ap().rearrange("(c p) -> p c", c=2))
              b3_s = cp.tile([1, 1], F32)
              nc.sync.dma_start(b3_s[:], b3.ap().unsqueeze(0))
              # pair-major logits: partition p holds rows 2p,2p+1 so the
              # stage-D strip activation writes it directly (no SBUF->SBUF
              # DMA); the output DMA un-reshapes via the DRAM-side AP
              logits_sb = cp.tile([npair, 2 * nj], F32)

              # tpfull: 3-deep manual stationary ring whose last partition
              # row permanently holds b1 (the K=33 bias trick lives in the
              # stationary, so it is filled only once, not per pair)
              TPB = 3
              tpfull = cp.tile([D + 1, TPB, 2, 2, H], F16)
              for t in range(TPB):
                  nc.gpsimd.dma_start(
                      tpfull[D : D + 1, t, :, :, :],
                      b1.ap().unsqueeze(0).broadcast_to([4, H]).unsqueeze(0),
                  )

              # ---------- stage A: tmp2xT[c, i, :] ----------
              sbA = None
              for n in range(D):
                  psA = ps.tile([rows, H], F32, tag="ac", bufs=3)
                  nc.tensor.matmul(psA[:], ziT_s[:], W1_s[:, n, :], start=True, stop=True)
                  if n % 2 == 0:
                      sbA = wp.tile([rows, 2, H], F16, tag="sa", bufs=6)
                      nc.vector.tensor_copy(sbA[:, 0, :], psA[:])
                  else:
                      nc.scalar.copy(sbA[:, 1, :], psA[:])
                      # SP/HWDGE; transpose to c-major on the DRAM-side AP
                      nc.sync.dma_start(
                          tmp2xT.ap()[n - 1 : n + 1, :, :].rearrange("c i h -> i c h"),
                          sbA[:],
                      )

              # ---------- main loop over i-pairs (software-pipelined) ----------
              # 3-stage pipeline: B(p), C(p-1), D(p-2).  Each cross-engine
              # handoff (PE->DVE/Act evict -> PE) gets a full pair of slack;
              # HW semaphore latency is far larger than the cost model's
              # 100ns, so depth, not speed, is what hides it.
              def tp_fetch(g):
                  # one gather covers two pairs (group g)
                  if g * 2 < npair:
                      nc.sync.dma_start(
                          tpfull[0:D, g % TPB, :, :, :],
                          tmp2xT.ap()[:, 4 * g : 4 * g + 4, :].rearrange(
                              "c (p i) h -> c p i h", p=2
                          ),
                      )

              def stage_B(p):
                  if p % 2 == 0:
                      tp_fetch(p // 2 + 2)  # prefetch two groups (4 pairs) ahead
                  tp = tpfull[:, (p // 2) % TPB, p % 2]
                  h1T = wp.tile([128, 4, 2 * nj], F16, tag="h1", bufs=5)
                  for i in range(2):
                      # quarters padded to 256 so each matmul output stays
                      # inside one 2KB PSUM bank (accumulation is per-bank)
                      psB = ps.tile([128, 4, 256], F32, tag="b", bufs=2)
                      for hc in range(4):
                          nc.tensor.matmul(
                              psB[:, hc, 0:nj],
                              tp[:, i, hc * 128 : (hc + 1) * 128],
                              zTx_s[:],
                              start=(hc % 2 == 0),
                              stop=(hc % 2 == 1),
                          )
                      # relu; bias already folded in via the K=33 ones row
                      nc.vector.tensor_scalar(
                          h1T[:, :, i * nj : (i + 1) * nj], psB[:, :, 0:nj],
                          0.0, None, ALU.max,
                      )
                  return h1T

              def stage_C(p, h1T):
                  h2T = wp.tile([128, 2, 2 * nj], F16, tag="h2", bufs=5)
                  for kc in range(2):
                      psC = ps.tile([128, 2 * nj], F32, tag="ac", bufs=3)
                      for hc in range(4):
                          nc.tensor.matmul(
                              psC[:],
                              W2_s[:, hc, kc * 128 : (kc + 1) * 128],
                              h1T[:, hc, :],
                              start=(hc == 0),
                              stop=(hc == 3),
                          )
                      nc.scalar.activation(
                          h2T[:, kc, :], psC[:], AF.Relu, bias=b2_s[:, kc : kc + 1]
                      )
                  return h2T

              def stage_D(p, h2T):
                  psD = ps.tile([1, 2 * nj], F32, tag="d", bufs=1)
                  nc.tensor.matmul(psD[:], W3_s[:, 0:1], h2T[:, 0, :], start=True, stop=False)
                  nc.tensor.matmul(psD[:], W3_s[:, 1:2], h2T[:, 1, :], start=False, stop=True)
                  # engines cannot write at partition base p, only DMA can
                  # shift partitions; Pool's software DGE is idle so the tiny
                  # copy is free
                  strip = wp.tile([1, 2 * nj], F32, tag="st")
                  nc.scalar.activation(strip[:], psD[:], AF.Identity, bias=b3_s[:])
                  nc.gpsimd.dma_start(logits_sb[p : p + 1, :], strip[:])

              # epilogue split at pair 32: engine partition bases must be
              # 0/32/64/96
              cmap_sb = cp.tile([npair, 2 * nj], F32)
              half_pairs = min(32, npair)
              logits_ov = logits_o.ap().rearrange("(p i) j -> p (i j)", i=2)
              cmap_ov = cmap_o.ap().rearrange("(p i) j -> p (i j)", i=2)

              def epilogue_half(h):
                  psl = slice(0, half_pairs) if h == 0 else slice(half_pairs, npair)
                  if psl.start >= psl.stop:
                      return
                  # logits dep is ready when emitted; cmap DMA goes on the Act
                  # queue right behind its sigmoid so SP's tp triggers for the
                  # second half are never blocked waiting on it
                  nc.sync.dma_start(logits_ov[psl, :], logits_sb[psl, :])
                  nc.scalar.activation(cmap_sb[psl, :], logits_sb[psl, :], AF.Sigmoid)
                  nc.scalar.dma_start(cmap_ov[psl, :], cmap_sb[psl, :])

              tp_fetch(0)
              tp_fetch(1)
              h1q, h2q = [], []

              def drain_c():
                  q, h1 = h1q.pop(0)
                  h2q.append((q, stage_C(q, h1)))

              def drain_d():
                  q, h2 = h2q.pop(0)
                  stage_D(q, h2)
                  if q == half_pairs - 1:
                      epilogue_half(0)

              for p in range(npair):
                  h1q.append((p, stage_B(p)))
                  if len(h1q) > 2:
                      drain_c()
                  if len(h2q) > 2:
                      drain_d()
              while h1q:
                  drain_c()
              while h2q:
                  drain_d()
              epilogue_half(1)

    nc.compile()
    return nc


def _active_idx(motif_mask):
    return [np.nonzero(np.asarray(motif_mask[b]) > 0.5)[0] for b in range(B)]


def _core_rows(idx_b, half):
    n = len(idx_b)
    h = (n + 1) // 2
    return idx_b[:h] if half == 0 else idx_b[h:]


def _in_maps(z, motif_mask, W1, b1, W2, b2, W3, b3, rows=ROWS, nj=NJ):
    z = np.ascontiguousarray(np.asarray(z, dtype=np.float32))
    motif_mask = np.asarray(motif_mask, dtype=np.float32)
    W1 = np.ascontiguousarray(np.asarray(W1, dtype=np.float16)).reshape(D * D, H)
    W2 = np.ascontiguousarray(np.asarray(W2, dtype=np.float32)).reshape(H, H // 2)
    W3 = np.ascontiguousarray(np.asarray(W3, dtype=np.float32)).reshape(H // 2, 1)
    b1 = np.ascontiguousarray(np.asarray(b1, dtype=np.float32)).reshape(H)
    b2 = np.ascontiguousarray(np.asarray(b2, dtype=np.float32)).reshape(H // 2)
    b3 = np.ascontiguousarray(np.asarray(b3, dtype=np.float32)).reshape(1)
    idx = _active_idx(motif_mask)
    maps = []
    for c in range(NCORES):
        b, half = divmod(c, 2)
        act = idx[b]
        n = len(act)
        rows_c = _core_rows(act, half)
        zb = z[b]  # (N, D)
        ziT = np.zeros((D, rows), np.float16)
        ziT[:, : len(rows_c)] = zb[rows_c].T.astype(np.float16)
        zTx = np.zeros((D + 1, nj), np.float32)
        zTx[:D, :n] = zb[act].T
        zTx[D, :n] = 1.0
        maps.append(
            {
                "ziT": ziT,
                "zTx": zTx,
                "W1": W1,
                "W2": W2,
                "W3": W3,
                "b1": b1,
                "b2": b2,
                "b3": b3,
            }
        )
    return maps


def kernel(z, motif_mask, residue_mask, W1, b1, W2, b2, W3, b3):
    global _cached_nc
    motif_np = np.asarray(motif_mask, dtype=np.float32)
    idx = _active_idx(motif_np)
    max_n = max(len(ix) for ix in idx)

    if max_n <= NJ:
        rows, nj, key = ROWS, NJ, 1
    else:
        rows, nj, key = 128, 256, "full"  # universal fallback, any mask fits
    if key not in _cached_nc:
        _cached_nc[key] = _build(rows=rows, nj=nj)
    nc = _cached_nc[key]

    maps = _in_maps(z, motif_mask, W1, b1, W2, b2, W3, b3, rows=rows, nj=nj)
    res = run_bass_kernel_spmd(nc, maps, list(range(NCORES)))

    # masked entries are exact: logit = 0, cmap = sigmoid(0) = 0.5
    logits = np.zeros((B, N, N), np.float32)
    cmap = np.full((B, N, N), 0.5, np.float32)
    for c in range(NCORES):
        b, half = divmod(c, 2)
        act = idx[b]
        n = len(act)
        rows_c = _core_rows(act, half)
        if len(rows_c) == 0:
            continue
        lg = res.results[c]["logits"][: len(rows_c), :n]
        cm = res.results[c]["cmap"][: len(rows_c), :n]
        logits[b][np.ix_(rows_c, act)] = lg
        cmap[b][np.ix_(rows_c, act)] = cm
    return cmap, logits
